# revision 30
# baseline (speedup 1.0000x reference)
"""Causal self-attention Trainium2 Bass kernel (fp8 DoubleRow pipeline).

Problem (hardcoded): x [4, 2048, 1024] f32, wq/wk/wv/wo [1024, 1024], biases
[1024]; out = causal_mha(x) @ wo + bo with 16 heads of dim 64.

Sharding over 8 NeuronCores: data parallel on batch (4) x tensor parallel on
heads (2 groups of 8 heads). Core c handles batch c//2 and head-group c%2.
Each core computes its partial out-projection; the host sums the two partials
per batch, divides by the operand prescale (32*32=1024) and adds the bias
terms (bo + bv @ wo).

Numerics: weights are prescaled x32 on the host and split into fp8e4m3
hi/lo pairs (w = (w_hi + w_lo)/32, error ~0.1%); x is split the same way.
All projection matmuls run in fp8 DoubleRow perf mode (2 contraction planes
per instruction at 0.5 cycles/row) with 3 product terms (hi*hi + lo*hi +
hi*lo). q/k are evicted to fp8 for the score matmuls (q-chunks >=1), and to
fp32r for the q-chunk-0 scores where few-entry softmax rows make logit noise
expensive. v is evicted as an fp8 hi/lo pair stored as the two DoubleRow
planes of the AV matmul (against a stride-0 broadcast P), making the 2-term
v reconstruction free. Scores for chunks >=1 use DoubleRow with both planes
stride-0 aliased to the same q/k data (exact 2x, absorbed into the exp
scale). P = exp(scale*s - 1) is written by the activation engine directly as
fp8e4m3 (max logit ~6.1 keeps exp(s-1) well under the 240 e4m3 max),
causally masked in-place by gpsimd affine_select on diagonal blocks, and the
ones-column of the v-hi plane accumulates the softmax normalizer. Softmax
normalization broadcasts 1/z across partitions with the gpsimd
partition_broadcast op. The out-projection runs in bf16.

Schedule: projections and attention are software-pipelined. Projection
psum-tile "pieces" of chunk sc+1 are emitted between the attention score
pairs of q-chunk sc, so the tensor engine's projection work fills the slack
while the activation engine (exp, the real bottleneck at ~21M softmax
elements) stays saturated. PSUM plan: aux tag (pq/pv/po, 2 x 1 bank), score
pairs (2 x 2 banks), ot accumulators (2 x 1 bank) = 8 banks.
"""

import numpy as np

N_HEADS = 16
DH = 64
N_CORES = 8
TP = 2  # head groups
WS = 32.0  # host-side weight prescale

_cache = {}
TRACE = False  # set by test harness to request an NTFF trace
last_result = None  # BassKernelResults of the most recent kernel() call


def _build(SEQ, D, DG, HPG, reps=1):
    """Build + schedule the per-core Bass program. DG = per-core qkv width,
    HPG = heads per core."""
    from contextlib import ExitStack

    import concourse.tile as tile
    from concourse import bacc, mybir

    F32 = mybir.dt.float32
    F32R = mybir.dt.float32r
    BF16 = mybir.dt.bfloat16
    F8 = mybir.dt.float8e4
    AF = mybir.ActivationFunctionType
    ALU = mybir.AluOpType
    DR = mybir.MatmulPerfMode.DoubleRow

    KO = D // 128  # contraction subtiles for the projections
    KP = KO // 2  # DoubleRow contraction pairs
    MQ = DG // 128  # qkv-dim subtiles
    SC = 512  # q/s chunk size
    NSC = SEQ // SC  # chunks
    NJ = SC // 128  # 128-blocks per chunk
    NSB = SEQ // 128  # s blocks total
    NO = D // 512  # out-proj column chunks
    VW = 80  # padded v row [v0..v63, ones, pad*15]; DR plane stride % 16 == 0

    nc = bacc.Bacc("TRN2", target_bir_lowering=False, debug=False)
    xh = nc.dram_tensor("xh", [D, SEQ], F8, kind="ExternalInput")
    xl = nc.dram_tensor("xl", [D, SEQ], F8, kind="ExternalInput")
    wqh = nc.dram_tensor("wqh", [D, DG], F8, kind="ExternalInput")
    wql = nc.dram_tensor("wql", [D, DG], F8, kind="ExternalInput")
    wkh = nc.dram_tensor("wkh", [D, DG], F8, kind="ExternalInput")
    wkl = nc.dram_tensor("wkl", [D, DG], F8, kind="ExternalInput")
    wvh = nc.dram_tensor("wvh", [D, DG], F8, kind="ExternalInput")
    wvl = nc.dram_tensor("wvl", [D, DG], F8, kind="ExternalInput")
    wob = nc.dram_tensor("wob", [DG, D], BF16, kind="ExternalInput")
    bq = nc.dram_tensor("bq", [DG], F32, kind="ExternalInput")
    bk = nc.dram_tensor("bk", [DG], F32, kind="ExternalInput")
    out = nc.dram_tensor("out", [SEQ, D], F32, kind="ExternalOutput")

    # psum score value = 2 (stride-0 alias) * WS^2 (operand scales) * 8 * s
    scale_fp8 = 1.0 / (2 * WS * WS * np.sqrt(DH))
    scale_f32r = 1.0 / (WS * WS * np.sqrt(DH))
    C_SHIFT = 1.0

    rep_range = range(reps)
    with tile.TileContext(nc) as tc, ExitStack() as ctx:
        res = ctx.enter_context(tc.tile_pool(name="res", bufs=1))
        qT8 = res.tile([128, MQ, SEQ], F8, tag="qT8", name="qT8")
        kT8 = res.tile([128, MQ, SEQ], F8, tag="kT8", name="kT8")
        qT0 = res.tile([128, MQ, SC], F32R, tag="qT0", name="qT0")
        kT0 = res.tile([128, MQ, SC], F32R, tag="kT0", name="kT0")
        vn = res.tile([128, NSB, HPG, 2, VW], F8, tag="vn", name="vn")

        # ones / zeros columns of the v planes (softmax normalizer)
        ones_nb = res.tile([128, NSB, HPG], F32, tag="ones_nb", name="ones_nb")
        nc.gpsimd.memset(ones_nb[:], 1.0)
        nc.vector.tensor_copy(vn[:, :, :, 0, DH], ones_nb[:])
        nc.gpsimd.memset(ones_nb[:], 0.0)
        nc.vector.tensor_copy(vn[:, :, :, 1, DH], ones_nb[:])

        cbias = res.tile([128, 1], F32, tag="cbias", name="cbias")
        nc.gpsimd.memset(cbias[:], -C_SHIFT)

        bq_sb = res.tile([128, MQ], F32, tag="bq_sb", name="bq_sb")
        bk_sb = res.tile([128, MQ], F32, tag="bk_sb", name="bk_sb")
        bq_big = res.tile([128, MQ, SC], F32, tag="bq_big", name="bq_big")
        bk_big = res.tile([128, MQ, SC], F32, tag="bk_big", name="bk_big")
        nc.sync.dma_start(bq_sb[:], bq.ap().rearrange("(m p) -> p m", p=128))
        nc.sync.dma_start(bk_sb[:], bk.ap().rearrange("(m p) -> p m", p=128))
        for big, sb_t in ((bq_big, bq_sb), (bk_big, bk_sb)):
            nc.gpsimd.memset(big[:], 0.0)
            for m in range(MQ):
                nc.vector.tensor_scalar_add(big[:, m, :], big[:, m, :], sb_t[:, m : m + 1])

        for _rep in rep_range:
            with ExitStack() as ph:
                wpool = ph.enter_context(tc.tile_pool(name="wpool", bufs=1))
                xpool = ph.enter_context(tc.tile_pool(name="xpool", bufs=2))
                ppool = ph.enter_context(tc.tile_pool(name="ppool", bufs=12))
                otsb = ph.enter_context(tc.tile_pool(name="otsb", bufs=4))
                wrk = ph.enter_context(tc.tile_pool(name="wrk", bufs=4))
                outp = ph.enter_context(tc.tile_pool(name="outp", bufs=3))
                psum = ph.enter_context(tc.tile_pool(name="psum", bufs=1, space="PSUM"))

                xh_r = xh.ap().rearrange("(ko p) s -> p ko s", p=128)
                xl_r = xl.ap().rearrange("(ko p) s -> p ko s", p=128)

                # single-descriptor loads ordered by first use (each dma_start
                # costs ~650ns of serialized SP issue time)
                xc0h = xpool.tile([128, KO, SC], F8, tag="xch", name="xch")
                nc.sync.dma_start(xc0h[:], xh_r[:, :, 0:SC])
                wq_h = wpool.tile([128, KO, DG], F8, tag="wq_h", name="wq_h")
                nc.sync.dma_start(wq_h[:], wqh.ap().rearrange("(ko p) n -> p ko n", p=128))
                xc0l = xpool.tile([128, KO, SC], F8, tag="xcl", name="xcl")
                nc.sync.dma_start(xc0l[:], xl_r[:, :, 0:SC])
                wq_l = wpool.tile([128, KO, DG], F8, tag="wq_l", name="wq_l")
                nc.sync.dma_start(wq_l[:], wql.ap().rearrange("(ko p) n -> p ko n", p=128))
                wk_h = wpool.tile([128, KO, DG], F8, tag="wk_h", name="wk_h")
                nc.sync.dma_start(wk_h[:], wkh.ap().rearrange("(ko p) n -> p ko n", p=128))
                wk_l = wpool.tile([128, KO, DG], F8, tag="wk_l", name="wk_l")
                nc.sync.dma_start(wk_l[:], wkl.ap().rearrange("(ko p) n -> p ko n", p=128))
                wv_h = wpool.tile([128, KO, DG], F8, tag="wv_h", name="wv_h")
                nc.sync.dma_start(wv_h[:], wvh.ap().rearrange("(ko p) n -> p ko n", p=128))
                wv_l = wpool.tile([128, KO, DG], F8, tag="wv_l", name="wv_l")
                nc.sync.dma_start(wv_l[:], wvl.ap().rearrange("(ko p) n -> p ko n", p=128))
                wo_sb = wpool.tile([128, MQ, D], BF16, tag="wo_sb", name="wo_sb")
                nc.sync.dma_start(wo_sb[:], wob.ap().rearrange("(m p) n -> p m n", p=128))

                def proj_pieces(sc):
                    """Generator: emits one psum-piece of chunk sc per yield,
                    ordered so attention on this chunk can start early:
                    [q m0, k m0, v sb0..3, q m1, k m1, q m2, k m2, q m3, k m3]."""
                    if sc == 0:
                        xch, xcl = xc0h, xc0l
                    else:
                        xch = xpool.tile([128, KO, SC], F8, tag="xch", name="xch")
                        nc.sync.dma_start(xch[:], xh_r[:, :, sc * SC : (sc + 1) * SC])
                        xcl = xpool.tile([128, KO, SC], F8, tag="xcl", name="xcl")
                        nc.sync.dma_start(xcl[:], xl_r[:, :, sc * SC : (sc + 1) * SC])
                    ssl = slice(sc * SC, (sc + 1) * SC)

                    def qk_piece(dst, wh_t, wl_t, b, dst0, m):
                        pq = psum.tile([128, SC], F32, tag="aux", name="pq", bufs=2)
                        msl = slice(m * 128, (m + 1) * 128)
                        terms = ((xch, wh_t), (xcl, wh_t), (xch, wl_t))
                        nt = len(terms)
                        for t, (xa, wa) in enumerate(terms):
                            for kp in range(KP):
                                nc.tensor.matmul(
                                    pq[:],
                                    wa[:, 2 * kp : 2 * kp + 2, msl],
                                    xa[:, 2 * kp : 2 * kp + 2, :],
                                    start=(t == 0 and kp == 0),
                                    stop=(t == nt - 1 and kp == KP - 1),
                                    perf_mode=DR,
                                )
                        nc.vector.tensor_tensor(dst[:, m, ssl], pq[:], b[:, m, :], ALU.add)
                        if sc == 0:
                            nc.vector.tensor_tensor(
                                dst0[:, m, :], pq[:], b[:, m, :], ALU.add
                            )

                    def v_piece(sb):
                        pv = psum.tile([128, DG], F32, tag="aux", name="pv", bufs=2)
                        bsl = slice(sb * 128, (sb + 1) * 128)
                        terms = ((xch, wv_h), (xcl, wv_h), (xch, wv_l))
                        nt = len(terms)
                        for t, (xa, wa) in enumerate(terms):
                            for kp in range(KP):
                                nc.tensor.matmul(
                                    pv[:],
                                    xa[:, 2 * kp : 2 * kp + 2, bsl],
                                    wa[:, 2 * kp : 2 * kp + 2, :],
                                    start=(t == 0 and kp == 0),
                                    stop=(t == nt - 1 and kp == KP - 1),
                                    perf_mode=DR,
                                )
                        blk = sc * NJ + sb
                        pv_r = pv[:].rearrange("p (h d) -> p h d", d=DH)
                        nc.vector.tensor_copy(vn[:, blk, :, 0, 0:DH], pv_r)
                        nc.vector.tensor_tensor(
                            vn[:, blk, :, 1, 0:DH],
                            pv_r,
                            vn[:, blk, :, 0, 0:DH],
                            ALU.subtract,
                        )

                    qargs = (qT8, wq_h, wq_l, bq_big, qT0)
                    kargs = (kT8, wk_h, wk_l, bk_big, kT0)
                    qk_piece(*qargs, 0)
                    yield
                    qk_piece(*kargs, 0)
                    yield
                    for sb in range(NJ):
                        v_piece(sb)
                        yield
                    for m in range(1, MQ):
                        qk_piece(*qargs, m)
                        yield
                        qk_piece(*kargs, m)
                        yield

                def pull(gen, n=1):
                    if gen is None:
                        return
                    for _ in range(n):
                        try:
                            next(gen)
                        except StopIteration:
                            return

                def emit_pair(qc, m, p):
                    """Scores + exp + causal-zeroing for pair p of section
                    (qc, m). Returns the pend entry for the AV stage."""
                    escale = scale_f32r if qc == 0 else scale_fp8
                    kb0 = 2 * p
                    qs_p = 128 * max(0, kb0 - qc * NJ)
                    wp = SC - qs_p
                    psl = slice(qs_p, SC)
                    st_a = psum.tile([128, 2, SC], F32, tag="st", name="st_a", bufs=2)
                    st_b = psum.tile([128, 2, SC], F32, tag="st", name="st_b", bufs=2)
                    for i in range(2):
                        kb = kb0 + i
                        ksl = slice(kb * 128, (kb + 1) * 128)
                        # per-block causal slice; exp still covers [qs_p:] so
                        # the odd block's [qs_p, qs_b) region holds stale psum
                        # exp'd to harmless values that affine_select zeroes
                        qs_b = 128 * max(0, kb - qc * NJ)
                        if qc == 0 and qs_b == 128 * 3:
                            qs_b = 128 * 2  # keep f32r width >= 256
                        wb = SC - qs_b
                        bsl = slice(qs_b, SC)
                        qvl = slice(qc * SC + qs_b, (qc + 1) * SC)
                        for hb, st in ((0, st_a), (1, st_b)):
                            hsl = slice(64 * hb, 64 * hb + 64)
                            if qc == 0:
                                nc.tensor.matmul(
                                    st[:, i, bsl],
                                    kT0[hsl, m, ksl],
                                    qT0[hsl, m, qs_b:SC],
                                    start=True,
                                    stop=True,
                                )
                            else:
                                nc.tensor.matmul(
                                    st[:, i, bsl],
                                    kT8[hsl, m, ksl]
                                    .unsqueeze(1)
                                    .broadcast_to([64, 2, 128]),
                                    qT8[hsl, m, qvl]
                                    .unsqueeze(1)
                                    .broadcast_to([64, 2, wb]),
                                    start=True,
                                    stop=True,
                                    perf_mode=DR,
                                )
                    pa_t = ppool.tile([128, 2, SC], F8, tag="pt", name="pa_t")
                    nc.scalar.activation(
                        pa_t[:, :, psl], st_a[:, :, psl], AF.Exp,
                        scale=escale, bias=cbias[:],
                    )
                    pb_t = ppool.tile([128, 2, SC], F8, tag="pt", name="pb_t")
                    nc.scalar.activation(
                        pb_t[:, :, psl], st_b[:, :, psl], AF.Exp,
                        scale=escale, bias=cbias[:],
                    )
                    # zero causally-invalid P on diagonal-chunk blocks
                    for i in range(2):
                        kb = kb0 + i
                        j = kb - qc * NJ
                        if j < 0:
                            continue
                        for pt in (pa_t, pb_t):
                            nc.gpsimd.affine_select(
                                out=pt[:, i, psl],
                                in_=pt[:, i, psl],
                                pattern=[[1, wp]],
                                compare_op=ALU.is_ge,
                                fill=0.0,
                                base=qs_p - 128 * j,
                                channel_multiplier=-1,
                            )
                    return (kb0, pa_t, pb_t)

                def outproj_pieces(qc, otc):
                    """Generator: one [128,512] out-projection tile per yield.
                    Deferred into later (activation-bound) q-chunks so the
                    tensor engine's slack there absorbs it."""
                    for n in range(NO):
                        for sb in range(NJ):
                            po = psum.tile([128, 512], F32, tag="aux", name="po", bufs=2)
                            for g in range(MQ):
                                nc.tensor.matmul(
                                    po[:],
                                    otc[:, g, sb * 128 : (sb + 1) * 128],
                                    wo_sb[:, g, n * 512 : (n + 1) * 512],
                                    start=(g == 0),
                                    stop=(g == MQ - 1),
                                )
                            outt = outp.tile([128, 512], F32, tag="outt", name="outt")
                            nc.vector.tensor_copy(outt[:], po[:])
                            r0 = qc * SC + sb * 128
                            nc.sync.dma_start(
                                out.ap()[r0 : r0 + 128, n * 512 : (n + 1) * 512],
                                outt[:],
                            )
                            yield

                # projection pieces are pulled just-in-time during the
                # PE-bound early q-chunks and eagerly in the activation-bound
                # late ones; out-projections are deferred to the last q-chunk
                queue = [(0, proj_pieces(0))]
                oqueue = []
                pulled = {0: 0}

                def pullq(n=1):
                    for _ in range(n):
                        while queue:
                            sc, g = queue[0]
                            try:
                                next(g)
                                pulled[sc] = pulled.get(sc, 0) + 1
                                break
                            except StopIteration:
                                queue.pop(0)
                        if not queue:
                            return

                def ensure(sc, n):
                    """Emit chunk sc's pieces up to index n (JIT dependency)."""
                    while queue and pulled.get(sc, 0) < n:
                        if queue[0][0] > sc:
                            return
                        pullq()

                def opull(n=1):
                    for _ in range(n):
                        while oqueue:
                            try:
                                next(oqueue[0])
                                break
                            except StopIteration:
                                oqueue.pop(0)
                        if not oqueue:
                            return

                def need(m):
                    # pieces of a chunk needed before section m: q/k of m0 +
                    # all v (6), plus q/k per further m
                    return 6 if m == 0 else 6 + 2 * m

                ensure(0, 2)  # q m0, k m0

                # stagger the two small early q-chunks: qc1's exp-heavy
                # sections fill qc0's tensor-bound stretches without a large
                # just-in-time projection burst at one boundary
                sections = [
                    (0, 0), (0, 1), (1, 0), (0, 2), (1, 1), (0, 3), (1, 2), (1, 3),
                ] + [(qc, m) for qc in (2, 3) for m in range(MQ)]
                carry = []
                otcs = {}
                for si, (qc, m) in enumerate(sections):
                    npair = (qc + 1) * NJ // 2
                    nkb = npair * 2
                    if m == 0:
                        otcs[qc] = otsb.tile([128, MQ, SC], BF16, tag="otc", name="otc")
                        if qc + 1 < NSC:
                            queue.append((qc + 1, proj_pieces(qc + 1)))
                    otc = otcs[qc]
                    ot_a = psum.tile([DH + 1, SC], F32, tag="ot", name="ot_a", bufs=2)
                    ot_b = psum.tile([DH + 1, SC], F32, tag="ot", name="ot_b", bufs=2)

                    def emit_av(ent):
                        kb0, pa_t, pb_t = ent
                        for i in range(2):
                            kb = kb0 + i
                            qs = 128 * max(0, kb - qc * NJ)
                            osl = slice(qs, SC)
                            w = SC - qs
                            for hb, pt in ((0, pa_t), (1, pb_t)):
                                nc.tensor.matmul(
                                    (ot_a, ot_b)[hb][:, osl],
                                    vn[:, kb, 2 * m + hb, :, 0 : DH + 1],
                                    pt[:, i, osl].unsqueeze(1).broadcast_to([128, 2, w]),
                                    start=(kb == 0),
                                    stop=(kb == nkb - 1),
                                    perf_mode=DR,
                                )

                    pend = []
                    p0 = len(carry)
                    pend.extend(carry)
                    carry = []
                    for p in range(p0, npair):
                        pend.append(emit_pair(qc, m, p))
                        if len(pend) > 3:
                            emit_av(pend.pop(0))
                        # eager pulls in the activation-bound late chunks; a
                        # light trickle during qc1 pre-spreads chunk 2
                        if qc == 1 and queue:
                            pullq()
                        elif qc >= 2:
                            if queue:
                                pullq()
                            else:
                                opull()
                    last_of_qc = m == MQ - 1
                    # AV below needs this chunk's v blocks
                    ensure(qc, 6)
                    # lookahead: emit the next section's first score pairs so
                    # the activation engine stays fed through the drain /
                    # normalize / out-projection stretch below
                    if si + 1 < len(sections):
                        nqc, nm = sections[si + 1]
                        ensure(nqc, need(nm))
                        depth = 3 if last_of_qc else 1
                        ncap = (nqc + 1) * NJ // 2
                        for p in range(min(depth, ncap)):
                            carry.append(emit_pair(nqc, nm, p))
                    for ent in pend:
                        emit_av(ent)
                    # normalize both heads: 1/z broadcast across partitions
                    r_rows = []
                    for ot_ps in (ot_a, ot_b):
                        r_row = wrk.tile([1, SC], F32, tag="r_row", name="r_row")
                        with nc.allow_low_precision(
                            reason="approx reciprocal; scales whole rows"
                        ):
                            nc.vector.reciprocal(r_row[:], ot_ps[DH : DH + 1, :])
                        r_rows.append(r_row)
                    r64s = []
                    for r_row in r_rows:
                        r64 = wrk.tile([64, SC], F32, tag="r64", name="r64")
                        nc.gpsimd.partition_broadcast(r64[:], r_row[:])
                        r64s.append(r64)
                    for hb in range(2):
                        nc.vector.tensor_tensor(
                            otc[64 * hb : 64 * hb + 64, m, :],
                            (ot_a, ot_b)[hb][0:DH, :],
                            r64s[hb][:],
                            ALU.mult,
                        )
                    if qc >= 2:
                        if queue:
                            pullq()
                        else:
                            opull()

                    if last_of_qc:
                        oqueue.append(outproj_pieces(qc, otc))
                        if si == len(sections) - 1:
                            while oqueue:
                                opull()

    nc.compile()
    return nc


REPS = 1  # >1 only for device-time measurement via wall-clock deltas


def _get_nc(SEQ, D, DG, HPG):
    key = (SEQ, D, DG, HPG, REPS)
    if key not in _cache:
        _cache[key] = _build(SEQ, D, DG, HPG, REPS)
    return _cache[key]


def _split8(a):
    """fp8e4m3 hi/lo split of an f32 array."""
    import ml_dtypes

    E4 = ml_dtypes.float8_e4m3
    a = np.ascontiguousarray(a, dtype=np.float32)
    hi = a.astype(E4)
    lo = (a - hi.astype(np.float32)).astype(E4)
    return hi, lo


def kernel(x, wq, bq, wk, bk, wv, bv, wo, bo):
    import ml_dtypes
    from concourse.bass_utils import run_bass_kernel_spmd

    BF = ml_dtypes.bfloat16

    x = np.asarray(x, dtype=np.float32)
    wq = np.asarray(wq, dtype=np.float32)
    wk = np.asarray(wk, dtype=np.float32)
    wv = np.asarray(wv, dtype=np.float32)
    wo = np.asarray(wo, dtype=np.float32)
    bq = np.asarray(bq, dtype=np.float32)
    bk = np.asarray(bk, dtype=np.float32)
    bv = np.asarray(bv, dtype=np.float32)
    bo = np.asarray(bo, dtype=np.float32)

    bsz, SEQ, D = x.shape
    DG = D // TP
    HPG = N_HEADS // TP
    assert bsz * TP == N_CORES

    nc = _get_nc(SEQ, D, DG, HPG)

    xs = [_split8(x[b].T) for b in range(bsz)]
    in_maps = []
    for c in range(N_CORES):
        b, g = c // TP, c % TP
        csl = slice(g * DG, (g + 1) * DG)
        wq_h, wq_l = _split8(WS * wq[:, csl])
        wk_h, wk_l = _split8(WS * wk[:, csl])
        wv_h, wv_l = _split8(WS * wv[:, csl])
        in_maps.append(
            {
                "xh": xs[b][0],
                "xl": xs[b][1],
                "wqh": wq_h,
                "wql": wq_l,
                "wkh": wk_h,
                "wkl": wk_l,
                "wvh": wv_h,
                "wvl": wv_l,
                "wob": np.ascontiguousarray(WS * wo[csl, :]).astype(BF),
                "bq": np.ascontiguousarray(WS * bq[csl]),
                "bk": np.ascontiguousarray(WS * bk[csl]),
            }
        )

    global last_result
    res = None
    for attempt in range(3):
        try:
            res = run_bass_kernel_spmd(
                nc, in_maps, core_ids=list(range(N_CORES)), trace=TRACE
            )
            break
        except Exception:
            # transient device errors (NRT_EXEC_UNIT_UNRECOVERABLE) appear when
            # a previous process's teardown races our startup; they clear after
            # a short recovery delay
            if attempt == 2:
                raise
            import time as _time

            _time.sleep(15)
    assert res is not None
    last_result = res

    # host combine: sum the TP partials, undo the x32 weight prescales,
    # add bias terms (bv @ wo + bo)
    bias = (bv @ wo + bo).astype(np.float32)
    outs = np.empty((bsz, SEQ, D), dtype=np.float32)
    inv = 1.0 / (WS * WS)
    for b in range(bsz):
        acc = res.results[b * TP]["out"].astype(np.float32).copy()
        for g in range(1, TP):
            acc += res.results[b * TP + g]["out"]
        outs[b] = acc * inv + bias[None, :]
    return outs


# revision 34
# speedup vs baseline: 1.0219x; 1.0219x over previous
"""Causal self-attention Trainium2 Bass kernel (fp8 DoubleRow pipeline).

Problem (hardcoded): x [4, 2048, 1024] f32, wq/wk/wv/wo [1024, 1024], biases
[1024]; out = causal_mha(x) @ wo + bo with 16 heads of dim 64.

Sharding over 8 NeuronCores: data parallel on batch (4) x tensor parallel on
heads (2 groups of 8 heads). Core c handles batch c//2 and head-group c%2.
Each core computes its partial out-projection; the host sums the two partials
per batch, divides by the operand prescale (32*32=1024) and adds the bias
terms (bo + bv @ wo).

Numerics: weights are prescaled x32 on the host and split into fp8e4m3
hi/lo pairs (w = (w_hi + w_lo)/32, error ~0.1%); x is split the same way.
All projection matmuls run in fp8 DoubleRow perf mode (2 contraction planes
per instruction at 0.5 cycles/row) with 3 product terms (hi*hi + lo*hi +
hi*lo). q/k are evicted to fp8 for the score matmuls (q-chunks >=1), and to
fp32r for the q-chunk-0 scores where few-entry softmax rows make logit noise
expensive. v is evicted as an fp8 hi/lo pair stored as the two DoubleRow
planes of the AV matmul (against a stride-0 broadcast P), making the 2-term
v reconstruction free. Scores for chunks >=1 use DoubleRow with both planes
stride-0 aliased to the same q/k data (exact 2x, absorbed into the exp
scale). P = exp(scale*s - 1) is written by the activation engine directly as
fp8e4m3 (max logit ~6.1 keeps exp(s-1) well under the 240 e4m3 max),
causally masked in-place by gpsimd affine_select on diagonal blocks, and the
ones-column of the v-hi plane accumulates the softmax normalizer. Softmax
normalization broadcasts 1/z across partitions with the gpsimd
partition_broadcast op. The out-projection runs in bf16.

Schedule: projections and attention are software-pipelined. Projection
psum-tile "pieces" of chunk sc+1 are emitted between the attention score
pairs of q-chunk sc, so the tensor engine's projection work fills the slack
while the activation engine (exp, the real bottleneck at ~21M softmax
elements) stays saturated. PSUM plan: aux tag (pq/pv/po, 2 x 1 bank), score
pairs (2 x 2 banks), ot accumulators (2 x 1 bank) = 8 banks.
"""

import numpy as np

N_HEADS = 16
DH = 64
N_CORES = 8
TP = 2  # head groups
WS = 32.0  # host-side weight prescale

_cache = {}
TRACE = False  # set by test harness to request an NTFF trace
last_result = None  # BassKernelResults of the most recent kernel() call


def _build(SEQ, D, DG, HPG, reps=1):
    """Build + schedule the per-core Bass program. DG = per-core qkv width,
    HPG = heads per core."""
    from contextlib import ExitStack

    import concourse.tile as tile
    from concourse import bacc, mybir

    F32 = mybir.dt.float32
    F32R = mybir.dt.float32r
    BF16 = mybir.dt.bfloat16
    F8 = mybir.dt.float8e4
    AF = mybir.ActivationFunctionType
    ALU = mybir.AluOpType
    DR = mybir.MatmulPerfMode.DoubleRow

    KO = D // 128  # contraction subtiles for the projections
    KP = KO // 2  # DoubleRow contraction pairs
    MQ = DG // 128  # qkv-dim subtiles
    SC = 512  # q/s chunk size
    NSC = SEQ // SC  # chunks
    NJ = SC // 128  # 128-blocks per chunk
    NSB = SEQ // 128  # s blocks total
    NO = D // 512  # out-proj column chunks
    VW = 80  # padded v row [v0..v63, ones, pad*15]; DR plane stride % 16 == 0

    nc = bacc.Bacc("TRN2", target_bir_lowering=False, debug=False)
    xh = nc.dram_tensor("xh", [D, SEQ], F8, kind="ExternalInput")
    xl = nc.dram_tensor("xl", [D, SEQ], F8, kind="ExternalInput")
    wqh = nc.dram_tensor("wqh", [D, DG], F8, kind="ExternalInput")
    wql = nc.dram_tensor("wql", [D, DG], F8, kind="ExternalInput")
    wkh = nc.dram_tensor("wkh", [D, DG], F8, kind="ExternalInput")
    wkl = nc.dram_tensor("wkl", [D, DG], F8, kind="ExternalInput")
    wvh = nc.dram_tensor("wvh", [D, DG], F8, kind="ExternalInput")
    wvl = nc.dram_tensor("wvl", [D, DG], F8, kind="ExternalInput")
    wob = nc.dram_tensor("wob", [DG, D], BF16, kind="ExternalInput")
    bq = nc.dram_tensor("bq", [DG], F32, kind="ExternalInput")
    bk = nc.dram_tensor("bk", [DG], F32, kind="ExternalInput")
    out = nc.dram_tensor("out", [SEQ, D], F32, kind="ExternalOutput")

    # psum score value = 2 (stride-0 alias) * WS^2 (operand scales) * 8 * s
    scale_fp8 = 1.0 / (2 * WS * WS * np.sqrt(DH))
    scale_f32r = 1.0 / (WS * WS * np.sqrt(DH))
    C_SHIFT = 1.0

    rep_range = range(reps)
    with tile.TileContext(nc) as tc, ExitStack() as ctx:
        res = ctx.enter_context(tc.tile_pool(name="res", bufs=1))
        qT8 = res.tile([128, MQ, SEQ], F8, tag="qT8", name="qT8")
        kT8 = res.tile([128, MQ, SEQ], F8, tag="kT8", name="kT8")
        qT0 = res.tile([128, MQ, SC], F32R, tag="qT0", name="qT0")
        kT0 = res.tile([128, MQ, SC], F32R, tag="kT0", name="kT0")
        vn = res.tile([128, NSB, HPG, 2, VW], F8, tag="vn", name="vn")

        # ones / zeros columns of the v planes (softmax normalizer)
        ones_nb = res.tile([128, NSB, HPG], F32, tag="ones_nb", name="ones_nb")
        nc.gpsimd.memset(ones_nb[:], 1.0)
        nc.vector.tensor_copy(vn[:, :, :, 0, DH], ones_nb[:])
        nc.gpsimd.memset(ones_nb[:], 0.0)
        nc.vector.tensor_copy(vn[:, :, :, 1, DH], ones_nb[:])

        cbias = res.tile([128, 1], F32, tag="cbias", name="cbias")
        nc.gpsimd.memset(cbias[:], -C_SHIFT)

        bq_sb = res.tile([128, MQ], F32, tag="bq_sb", name="bq_sb")
        bk_sb = res.tile([128, MQ], F32, tag="bk_sb", name="bk_sb")
        bq_big = res.tile([128, MQ, SC], F32, tag="bq_big", name="bq_big")
        bk_big = res.tile([128, MQ, SC], F32, tag="bk_big", name="bk_big")
        nc.sync.dma_start(bq_sb[:], bq.ap().rearrange("(m p) -> p m", p=128))
        nc.sync.dma_start(bk_sb[:], bk.ap().rearrange("(m p) -> p m", p=128))
        for big, sb_t in ((bq_big, bq_sb), (bk_big, bk_sb)):
            nc.gpsimd.memset(big[:], 0.0)
            for m in range(MQ):
                nc.vector.tensor_scalar_add(big[:, m, :], big[:, m, :], sb_t[:, m : m + 1])

        for _rep in rep_range:
            with ExitStack() as ph:
                wpool = ph.enter_context(tc.tile_pool(name="wpool", bufs=1))
                xpool = ph.enter_context(tc.tile_pool(name="xpool", bufs=2))
                ppool = ph.enter_context(tc.tile_pool(name="ppool", bufs=12))
                otsb = ph.enter_context(tc.tile_pool(name="otsb", bufs=4))
                wrk = ph.enter_context(tc.tile_pool(name="wrk", bufs=4))
                outp = ph.enter_context(tc.tile_pool(name="outp", bufs=3))
                psum = ph.enter_context(tc.tile_pool(name="psum", bufs=1, space="PSUM"))

                xh_r = xh.ap().rearrange("(ko p) s -> p ko s", p=128)
                xl_r = xl.ap().rearrange("(ko p) s -> p ko s", p=128)

                # single-descriptor loads ordered by first use (each dma_start
                # costs ~650ns of serialized SP issue time)
                xc0h = xpool.tile([128, KO, SC], F8, tag="xch", name="xch")
                nc.sync.dma_start(xc0h[:], xh_r[:, :, 0:SC])
                wq_h = wpool.tile([128, KO, DG], F8, tag="wq_h", name="wq_h")
                nc.sync.dma_start(wq_h[:], wqh.ap().rearrange("(ko p) n -> p ko n", p=128))
                xc0l = xpool.tile([128, KO, SC], F8, tag="xcl", name="xcl")
                nc.sync.dma_start(xc0l[:], xl_r[:, :, 0:SC])
                wq_l = wpool.tile([128, KO, DG], F8, tag="wq_l", name="wq_l")
                nc.sync.dma_start(wq_l[:], wql.ap().rearrange("(ko p) n -> p ko n", p=128))
                wk_h = wpool.tile([128, KO, DG], F8, tag="wk_h", name="wk_h")
                nc.sync.dma_start(wk_h[:], wkh.ap().rearrange("(ko p) n -> p ko n", p=128))
                wk_l = wpool.tile([128, KO, DG], F8, tag="wk_l", name="wk_l")
                nc.sync.dma_start(wk_l[:], wkl.ap().rearrange("(ko p) n -> p ko n", p=128))
                wv_h = wpool.tile([128, KO, DG], F8, tag="wv_h", name="wv_h")
                nc.sync.dma_start(wv_h[:], wvh.ap().rearrange("(ko p) n -> p ko n", p=128))
                wv_l = wpool.tile([128, KO, DG], F8, tag="wv_l", name="wv_l")
                nc.sync.dma_start(wv_l[:], wvl.ap().rearrange("(ko p) n -> p ko n", p=128))
                wo_sb = wpool.tile([128, MQ, D], BF16, tag="wo_sb", name="wo_sb")
                nc.sync.dma_start(wo_sb[:], wob.ap().rearrange("(m p) n -> p m n", p=128))

                def proj_pieces(sc):
                    """Generator: emits one psum-piece of chunk sc per yield,
                    ordered so attention on this chunk can start early:
                    [q m0, k m0, v sb0..3, q m1, k m1, q m2, k m2, q m3, k m3]."""
                    if sc == 0:
                        xch, xcl = xc0h, xc0l
                    else:
                        xch = xpool.tile([128, KO, SC], F8, tag="xch", name="xch")
                        nc.sync.dma_start(xch[:], xh_r[:, :, sc * SC : (sc + 1) * SC])
                        xcl = xpool.tile([128, KO, SC], F8, tag="xcl", name="xcl")
                        nc.sync.dma_start(xcl[:], xl_r[:, :, sc * SC : (sc + 1) * SC])
                    ssl = slice(sc * SC, (sc + 1) * SC)

                    def qk_piece(dst, wh_t, wl_t, b, dst0, m):
                        pq = psum.tile([128, SC], F32, tag="aux", name="pq", bufs=2)
                        msl = slice(m * 128, (m + 1) * 128)
                        terms = ((xch, wh_t), (xcl, wh_t), (xch, wl_t))
                        nt = len(terms)
                        for t, (xa, wa) in enumerate(terms):
                            for kp in range(KP):
                                nc.tensor.matmul(
                                    pq[:],
                                    wa[:, 2 * kp : 2 * kp + 2, msl],
                                    xa[:, 2 * kp : 2 * kp + 2, :],
                                    start=(t == 0 and kp == 0),
                                    stop=(t == nt - 1 and kp == KP - 1),
                                    perf_mode=DR,
                                )
                        nc.vector.tensor_tensor(dst[:, m, ssl], pq[:], b[:, m, :], ALU.add)
                        if sc == 0:
                            nc.vector.tensor_tensor(
                                dst0[:, m, :], pq[:], b[:, m, :], ALU.add
                            )

                    def v_piece(sb):
                        pv = psum.tile([128, DG], F32, tag="aux", name="pv", bufs=2)
                        bsl = slice(sb * 128, (sb + 1) * 128)
                        terms = ((xch, wv_h), (xcl, wv_h), (xch, wv_l))
                        nt = len(terms)
                        for t, (xa, wa) in enumerate(terms):
                            for kp in range(KP):
                                nc.tensor.matmul(
                                    pv[:],
                                    xa[:, 2 * kp : 2 * kp + 2, bsl],
                                    wa[:, 2 * kp : 2 * kp + 2, :],
                                    start=(t == 0 and kp == 0),
                                    stop=(t == nt - 1 and kp == KP - 1),
                                    perf_mode=DR,
                                )
                        blk = sc * NJ + sb
                        pv_r = pv[:].rearrange("p (h d) -> p h d", d=DH)
                        nc.vector.tensor_copy(vn[:, blk, :, 0, 0:DH], pv_r)
                        nc.vector.tensor_tensor(
                            vn[:, blk, :, 1, 0:DH],
                            pv_r,
                            vn[:, blk, :, 0, 0:DH],
                            ALU.subtract,
                        )

                    qargs = (qT8, wq_h, wq_l, bq_big, qT0)
                    kargs = (kT8, wk_h, wk_l, bk_big, kT0)
                    qk_piece(*qargs, 0)
                    yield
                    qk_piece(*kargs, 0)
                    yield
                    for sb in range(NJ):
                        v_piece(sb)
                        yield
                    for m in range(1, MQ):
                        qk_piece(*qargs, m)
                        yield
                        qk_piece(*kargs, m)
                        yield

                def pull(gen, n=1):
                    if gen is None:
                        return
                    for _ in range(n):
                        try:
                            next(gen)
                        except StopIteration:
                            return

                def emit_pair(qc, m, p):
                    """Scores + exp + causal-zeroing for pair p of section
                    (qc, m). Returns the pend entry for the AV stage."""
                    escale = scale_f32r if qc == 0 else scale_fp8
                    kb0 = 2 * p
                    qs_p = 128 * max(0, kb0 - qc * NJ)
                    wp = SC - qs_p
                    psl = slice(qs_p, SC)
                    st_a = psum.tile([128, 2, SC], F32, tag="st", name="st_a", bufs=2)
                    st_b = psum.tile([128, 2, SC], F32, tag="st", name="st_b", bufs=2)
                    for i in range(2):
                        kb = kb0 + i
                        ksl = slice(kb * 128, (kb + 1) * 128)
                        # per-block causal slice; exp still covers [qs_p:] so
                        # the odd block's [qs_p, qs_b) region holds stale psum
                        # exp'd to harmless values that affine_select zeroes
                        qs_b = 128 * max(0, kb - qc * NJ)
                        if qc == 0 and qs_b == 128 * 3:
                            qs_b = 128 * 2  # keep f32r width >= 256
                        wb = SC - qs_b
                        bsl = slice(qs_b, SC)
                        qvl = slice(qc * SC + qs_b, (qc + 1) * SC)
                        for hb, st in ((0, st_a), (1, st_b)):
                            hsl = slice(64 * hb, 64 * hb + 64)
                            if qc == 0:
                                nc.tensor.matmul(
                                    st[:, i, bsl],
                                    kT0[hsl, m, ksl],
                                    qT0[hsl, m, qs_b:SC],
                                    start=True,
                                    stop=True,
                                )
                            else:
                                nc.tensor.matmul(
                                    st[:, i, bsl],
                                    kT8[hsl, m, ksl]
                                    .unsqueeze(1)
                                    .broadcast_to([64, 2, 128]),
                                    qT8[hsl, m, qvl]
                                    .unsqueeze(1)
                                    .broadcast_to([64, 2, wb]),
                                    start=True,
                                    stop=True,
                                    perf_mode=DR,
                                )
                    pa_t = ppool.tile([128, 2, SC], F8, tag="pt", name="pa_t")
                    nc.scalar.activation(
                        pa_t[:, :, psl], st_a[:, :, psl], AF.Exp,
                        scale=escale, bias=cbias[:],
                    )
                    pb_t = ppool.tile([128, 2, SC], F8, tag="pt", name="pb_t")
                    nc.scalar.activation(
                        pb_t[:, :, psl], st_b[:, :, psl], AF.Exp,
                        scale=escale, bias=cbias[:],
                    )
                    # zero causally-invalid P on diagonal-chunk blocks
                    for i in range(2):
                        kb = kb0 + i
                        j = kb - qc * NJ
                        if j < 0:
                            continue
                        for pt in (pa_t, pb_t):
                            nc.gpsimd.affine_select(
                                out=pt[:, i, psl],
                                in_=pt[:, i, psl],
                                pattern=[[1, wp]],
                                compare_op=ALU.is_ge,
                                fill=0.0,
                                base=qs_p - 128 * j,
                                channel_multiplier=-1,
                            )
                    return (kb0, pa_t, pb_t)

                def outproj_pieces(qc, otc):
                    """Generator: one [128,512] out-projection tile per yield.
                    Deferred into later (activation-bound) q-chunks so the
                    tensor engine's slack there absorbs it. The final chunk's
                    pieces run after all score pairs, so they can use the
                    idle score psum banks for a deeper pipeline."""
                    ptag = "st" if qc == NSC - 1 else "aux"
                    for n in range(NO):
                        for sb in range(NJ):
                            po = psum.tile([128, 512], F32, tag=ptag, name="po", bufs=2)
                            for g in range(MQ):
                                nc.tensor.matmul(
                                    po[:],
                                    otc[:, g, sb * 128 : (sb + 1) * 128],
                                    wo_sb[:, g, n * 512 : (n + 1) * 512],
                                    start=(g == 0),
                                    stop=(g == MQ - 1),
                                )
                            outt = outp.tile([128, 512], F32, tag="outt", name="outt")
                            nc.vector.tensor_copy(outt[:], po[:])
                            r0 = qc * SC + sb * 128
                            nc.sync.dma_start(
                                out.ap()[r0 : r0 + 128, n * 512 : (n + 1) * 512],
                                outt[:],
                            )
                            yield

                # projection pieces are pulled just-in-time during the
                # PE-bound early q-chunks and eagerly in the activation-bound
                # late ones; out-projections are deferred to the last q-chunk
                queue = [(0, proj_pieces(0))]
                oqueue = []
                pulled = {0: 0}

                def pullq(n=1):
                    for _ in range(n):
                        while queue:
                            sc, g = queue[0]
                            try:
                                next(g)
                                pulled[sc] = pulled.get(sc, 0) + 1
                                break
                            except StopIteration:
                                queue.pop(0)
                        if not queue:
                            return

                def ensure(sc, n):
                    """Emit chunk sc's pieces up to index n (JIT dependency)."""
                    while queue and pulled.get(sc, 0) < n:
                        if queue[0][0] > sc:
                            return
                        pullq()

                def opull(n=1):
                    for _ in range(n):
                        while oqueue:
                            try:
                                next(oqueue[0])
                                break
                            except StopIteration:
                                oqueue.pop(0)
                        if not oqueue:
                            return

                def need(m):
                    # pieces of a chunk needed before section m: q/k of m0 +
                    # all v (6), plus q/k per further m
                    return 6 if m == 0 else 6 + 2 * m

                ensure(0, 2)  # q m0, k m0

                sections = [(qc, m) for qc in range(NSC) for m in range(MQ)]
                carry = []
                otcs = {}
                for si, (qc, m) in enumerate(sections):
                    npair = (qc + 1) * NJ // 2
                    nkb = npair * 2
                    if m == 0:
                        otcs[qc] = otsb.tile([128, MQ, SC], BF16, tag="otc", name="otc")
                        if qc + 1 < NSC:
                            queue.append((qc + 1, proj_pieces(qc + 1)))
                    otc = otcs[qc]
                    ot_a = psum.tile([DH + 1, SC], F32, tag="ot", name="ot_a", bufs=2)
                    ot_b = psum.tile([DH + 1, SC], F32, tag="ot", name="ot_b", bufs=2)

                    def emit_av(ent):
                        kb0, pa_t, pb_t = ent
                        if qc >= 2:
                            # late rows: 1-term v, DoubleRow planes = the two
                            # k-blocks of the pair (softmax averaging washes
                            # out the v-lo term there)
                            qs = 128 * max(0, kb0 - qc * NJ)
                            osl = slice(qs, SC)
                            w = SC - qs
                            for hb, pt in ((0, pa_t), (1, pb_t)):
                                nc.tensor.matmul(
                                    (ot_a, ot_b)[hb][:, osl],
                                    vn[:, kb0 : kb0 + 2, 2 * m + hb, 0, 0 : DH + 1],
                                    pt[:, :, osl],
                                    start=(kb0 == 0),
                                    stop=(kb0 + 2 == nkb),
                                    perf_mode=DR,
                                )
                            return
                        for i in range(2):
                            kb = kb0 + i
                            qs = 128 * max(0, kb - qc * NJ)
                            osl = slice(qs, SC)
                            w = SC - qs
                            for hb, pt in ((0, pa_t), (1, pb_t)):
                                nc.tensor.matmul(
                                    (ot_a, ot_b)[hb][:, osl],
                                    vn[:, kb, 2 * m + hb, :, 0 : DH + 1],
                                    pt[:, i, osl].unsqueeze(1).broadcast_to([128, 2, w]),
                                    start=(kb == 0),
                                    stop=(kb == nkb - 1),
                                    perf_mode=DR,
                                )

                    pend = []
                    p0 = len(carry)
                    pend.extend(carry)
                    carry = []
                    for p in range(p0, npair):
                        pend.append(emit_pair(qc, m, p))
                        if len(pend) > 3:
                            emit_av(pend.pop(0))
                        # eager pulls only in the activation-bound late chunks
                        if qc >= 2:
                            if queue:
                                pullq()
                            else:
                                opull()
                    last_of_qc = m == MQ - 1
                    # AV below needs this chunk's v blocks
                    ensure(qc, 6)
                    # lookahead: emit the next section's first score pairs so
                    # the activation engine stays fed through the drain /
                    # normalize / out-projection stretch below
                    if si + 1 < len(sections):
                        nqc, nm = sections[si + 1]
                        ensure(nqc, need(nm))
                        depth = 3 if last_of_qc else 1
                        ncap = (nqc + 1) * NJ // 2
                        for p in range(min(depth, ncap)):
                            carry.append(emit_pair(nqc, nm, p))
                    for ent in pend:
                        emit_av(ent)
                    # normalize both heads: 1/z broadcast across partitions
                    r_rows = []
                    for ot_ps in (ot_a, ot_b):
                        r_row = wrk.tile([1, SC], F32, tag="r_row", name="r_row")
                        with nc.allow_low_precision(
                            reason="approx reciprocal; scales whole rows"
                        ):
                            nc.vector.reciprocal(r_row[:], ot_ps[DH : DH + 1, :])
                        r_rows.append(r_row)
                    r64s = []
                    for r_row in r_rows:
                        r64 = wrk.tile([64, SC], F32, tag="r64", name="r64")
                        nc.gpsimd.partition_broadcast(r64[:], r_row[:])
                        r64s.append(r64)
                    for hb in range(2):
                        nc.vector.tensor_tensor(
                            otc[64 * hb : 64 * hb + 64, m, :],
                            (ot_a, ot_b)[hb][0:DH, :],
                            r64s[hb][:],
                            ALU.mult,
                        )
                    if qc >= 2:
                        if queue:
                            pullq()
                        else:
                            opull()

                    if last_of_qc:
                        oqueue.append(outproj_pieces(qc, otc))
                        if si == len(sections) - 1:
                            while oqueue:
                                opull()

    nc.compile()
    return nc


REPS = 1  # >1 only for device-time measurement via wall-clock deltas


def _get_nc(SEQ, D, DG, HPG):
    key = (SEQ, D, DG, HPG, REPS)
    if key not in _cache:
        _cache[key] = _build(SEQ, D, DG, HPG, REPS)
    return _cache[key]


def _split8(a):
    """fp8e4m3 hi/lo split of an f32 array."""
    import ml_dtypes

    E4 = ml_dtypes.float8_e4m3
    a = np.ascontiguousarray(a, dtype=np.float32)
    hi = a.astype(E4)
    lo = (a - hi.astype(np.float32)).astype(E4)
    return hi, lo


def kernel(x, wq, bq, wk, bk, wv, bv, wo, bo):
    import ml_dtypes
    from concourse.bass_utils import run_bass_kernel_spmd

    BF = ml_dtypes.bfloat16

    x = np.asarray(x, dtype=np.float32)
    wq = np.asarray(wq, dtype=np.float32)
    wk = np.asarray(wk, dtype=np.float32)
    wv = np.asarray(wv, dtype=np.float32)
    wo = np.asarray(wo, dtype=np.float32)
    bq = np.asarray(bq, dtype=np.float32)
    bk = np.asarray(bk, dtype=np.float32)
    bv = np.asarray(bv, dtype=np.float32)
    bo = np.asarray(bo, dtype=np.float32)

    bsz, SEQ, D = x.shape
    DG = D // TP
    HPG = N_HEADS // TP
    assert bsz * TP == N_CORES

    nc = _get_nc(SEQ, D, DG, HPG)

    xs = [_split8(x[b].T) for b in range(bsz)]
    in_maps = []
    for c in range(N_CORES):
        b, g = c // TP, c % TP
        csl = slice(g * DG, (g + 1) * DG)
        wq_h, wq_l = _split8(WS * wq[:, csl])
        wk_h, wk_l = _split8(WS * wk[:, csl])
        wv_h, wv_l = _split8(WS * wv[:, csl])
        in_maps.append(
            {
                "xh": xs[b][0],
                "xl": xs[b][1],
                "wqh": wq_h,
                "wql": wq_l,
                "wkh": wk_h,
                "wkl": wk_l,
                "wvh": wv_h,
                "wvl": wv_l,
                "wob": np.ascontiguousarray(WS * wo[csl, :]).astype(BF),
                "bq": np.ascontiguousarray(WS * bq[csl]),
                "bk": np.ascontiguousarray(WS * bk[csl]),
            }
        )

    global last_result
    res = None
    for attempt in range(3):
        try:
            res = run_bass_kernel_spmd(
                nc, in_maps, core_ids=list(range(N_CORES)), trace=TRACE
            )
            break
        except Exception:
            # transient device errors (NRT_EXEC_UNIT_UNRECOVERABLE) appear when
            # a previous process's teardown races our startup; they clear after
            # a short recovery delay
            if attempt == 2:
                raise
            import time as _time

            _time.sleep(15)
    assert res is not None
    last_result = res

    # host combine: sum the TP partials, undo the x32 weight prescales,
    # add bias terms (bv @ wo + bo)
    bias = (bv @ wo + bo).astype(np.float32)
    outs = np.empty((bsz, SEQ, D), dtype=np.float32)
    inv = 1.0 / (WS * WS)
    for b in range(bsz):
        acc = res.results[b * TP]["out"].astype(np.float32).copy()
        for g in range(1, TP):
            acc += res.results[b * TP + g]["out"]
        outs[b] = acc * inv + bias[None, :]
    return outs
